# revision 1
# baseline (speedup 1.0000x reference)
"""Trainium2 Bass kernel for the Cut+Balance loss.

loss = sum_i numer_i / Gamma_i + sum_i (colsum(Y)_i - N/G)^2
  where B = Y^T A  (G x N),
        Gamma_i = sum_m B[i, m]
        numer_i = sum_m B[i, m] * (1 - Y[m, i]) = Gamma_i - sum_m B[i,m] Y[m,i]

Strategy (8 NeuronCores, row-sharded A):
  - Each core owns 2048 rows of A (128 MB) and computes the local
    B_c = Yl^T A_c contribution entirely in PSUM using fp32r matmuls
    (full-rate fp32 streaming; PSUM accumulates fp32). fp32r outputs
    must sit at PSUM partition offset 0, so the N=16384 columns are
    processed in four quarter-passes of 8 column-tiles x 512 (one psum
    bank each), accumulated over the 16 row-blocks of 128 rows.
  - A DMA covers two row-blocks x 4096 cols = 4 MB of 16 KB-contiguous
    rows -> near-peak HBM bandwidth (roofline ~358 GB/s/core ~ 360 us).
  - Per bank, VectorE reduces psum rows to Gamma partials and (via
    tensor_tensor_reduce against a host-packed Y^T layout) to
    sum_m B[i,m] Y[m,i] partials.
  - Host sums the tiny per-core partials and adds the Y-only balance
    term.
"""

import sys

if "/opt/trn_rl_repo" not in sys.path:
    sys.path.insert(0, "/opt/trn_rl_repo")

import numpy as np

N = 16384
G = 16
NC = 8
R = N // NC            # 2048 rows of A per core
KT = R // 128          # 16 row-blocks per core
QP = 4                 # column quarter-passes
CN = N // QP           # 4096 columns per pass
JT = CN // 512         # 8 column tiles of 512 per pass (one psum bank each)
K2 = KT // 2           # 8 paired-row-block DMAs per pass

_NC_CACHE = None
last_results = None    # BassKernelResults of the most recent run


def _build():
    import concourse.mybir as mybir
    from concourse.bacc import Bacc
    from concourse.bass import MemorySpace, ds
    from concourse.tile import TileContext

    f32 = mybir.dt.float32
    f32r = mybir.dt.float32r

    nc = Bacc(trn_type="TRN2")
    a_d = nc.declare_dram_parameter("A", [R, N], f32r, isOutput=False)
    yl_d = nc.declare_dram_parameter("Ylp", [128, KT, G], f32r, isOutput=False)
    yt_d = nc.declare_dram_parameter("YTp", [128, QP, JT, 512], f32, isOutput=False)
    out_d = nc.declare_dram_parameter("out", [128, 2 * QP * JT], f32, isOutput=True)

    with TileContext(nc) as tc:
        with (
            tc.tile_pool(name="const", bufs=1) as cpool,
            tc.tile_pool(name="abuf", bufs=6) as apool,
            tc.tile_pool(name="scr", bufs=2) as spool,
            tc.tile_pool(name="psum", bufs=1, space=MemorySpace.PSUM) as ppool,
        ):
            yl = cpool.tile([128, KT, G], f32r)
            nc.sync.dma_start(out=yl, in_=yl_d[:])
            # YTp rows 0..15 hold Y^T; pass indexed on the free dim so all
            # DVE operands share partition base 0 (ISA requirement).
            yt = cpool.tile([128, QP, JT, 512], f32)
            nc.sync.dma_start(out=yt, in_=yt_d[:])
            gs = cpool.tile([128, 2 * QP * JT], f32)

            psums = [
                ppool.tile([128, 512], f32, name=f"psum{b}") for b in range(JT)
            ]

            for p in range(QP):
                for k in range(KT):
                    ak = apool.tile([128, CN], f32r)
                    nc.sync.dma_start(
                        out=ak, in_=a_d[ds(k * 128, 128), ds(p * CN, CN)]
                    )
                    for jj in range(JT):
                        nc.tensor.matmul(
                            psums[jj][ds(0, G), :],
                            yl[:, k, :],
                            ak[:, ds(512 * jj, 512)],
                            start=(k == 0),
                            stop=(k == KT - 1),
                        )

                # per-bank reductions of psum rows 0..15
                for jj in range(JT):
                    nc.vector.tensor_reduce(
                        out=gs[ds(0, G), ds(p * JT + jj, 1)],
                        in_=psums[jj][ds(0, G), :],
                        axis=mybir.AxisListType.X,
                        op=mybir.AluOpType.add,
                    )
                    # (tensor_tensor_reduce faults at runtime on HW here;
                    # use separate multiply + reduce instead)
                    scratch = spool.tile([128, 512], f32)
                    nc.vector.tensor_mul(
                        scratch[ds(0, G), :],
                        psums[jj][ds(0, G), :],
                        yt[ds(0, G), p, jj, :],
                    )
                    nc.vector.tensor_reduce(
                        out=gs[ds(0, G), ds(QP * JT + p * JT + jj, 1)],
                        in_=scratch[ds(0, G), :],
                        axis=mybir.AxisListType.X,
                        op=mybir.AluOpType.add,
                    )

            nc.sync.dma_start(out=out_d[:], in_=gs)

    nc.finalize()  # Bacc: runs wait-splitting (generate_event_semaphores) + reg alloc
    return nc


def _get_nc():
    global _NC_CACHE
    if _NC_CACHE is None:
        _NC_CACHE = _build()
    return _NC_CACHE


def _pack_inputs(Y, A):
    """Host-side packed layouts so the device does zero reshuffling."""
    # Ylp[c][p, k, i] = Y[c*R + k*128 + p, i]  (matmul lhsT per row-block)
    ylp = Y.reshape(NC, KT, 128, G).transpose(0, 2, 1, 3).copy()
    # YTp[i, p, jj, f] = Y[(p*JT + jj)*512 + f, i]; rows >= G zero.
    yt3 = Y.reshape(QP, JT, 512, G)  # [p, jj, f, i]
    ytp = np.zeros((128, QP, JT, 512), dtype=np.float32)
    ytp[:G] = yt3.transpose(3, 0, 1, 2)  # [i, p, jj, f]
    in_maps = [
        {"A": np.ascontiguousarray(A[c * R : (c + 1) * R]), "Ylp": ylp[c], "YTp": ytp}
        for c in range(NC)
    ]
    return in_maps


def kernel(Y, A, _trace=False, _trace_kwargs=None):
    global last_results
    Y = np.asarray(Y, dtype=np.float32)
    A = np.asarray(A, dtype=np.float32)
    assert Y.shape == (N, G) and A.shape == (N, N)

    from concourse.bass_utils import run_bass_kernel_spmd

    in_maps = _pack_inputs(Y, A)
    res = run_bass_kernel_spmd(
        _get_nc(),
        in_maps,
        core_ids=list(range(NC)),
        trace=_trace,
        **(_trace_kwargs or {}),
    )
    last_results = res

    g_total = np.zeros(G, dtype=np.float64)
    s_total = np.zeros(G, dtype=np.float64)
    for c in range(NC):
        o = np.asarray(res.results[c]["out"], dtype=np.float64)  # [128, 2*QP*JT]
        rows = o[:G]
        g_total += rows[:, : QP * JT].sum(axis=1)
        s_total += rows[:, QP * JT :].sum(axis=1)

    gamma = g_total
    numer = gamma - s_total
    cut = float(np.sum(numer / gamma))
    col = Y.sum(axis=0, dtype=np.float64)
    balance = float(np.sum((col - N / G) ** 2))
    return np.float32(cut + balance)



# revision 3
# speedup vs baseline: 1.0417x; 1.0417x over previous
"""Trainium2 Bass kernel for the Cut+Balance loss.

loss = sum_i numer_i / Gamma_i + sum_i (colsum(Y)_i - N/G)^2
  where B = Y^T A  (G x N),
        Gamma_i = sum_m B[i, m]
        numer_i = sum_m B[i, m] * (1 - Y[m, i]) = Gamma_i - sum_m B[i,m] Y[m,i]

Strategy (8 NeuronCores, row-sharded A):
  - Each core owns 2048 rows of A (128 MB) and computes the local
    B_c = Yl^T A_c contribution entirely in PSUM using fp32r matmuls
    (full-rate fp32 streaming; PSUM accumulates fp32). fp32r outputs
    must sit at PSUM partition offset 0, so the N=16384 columns are
    processed in four quarter-passes of 8 column-tiles x 512 (one psum
    bank each), accumulated over the 16 row-blocks of 128 rows.
  - A DMA covers one row-block x 4096 cols = 2 MB of 16 KB-contiguous
    rows -> near-peak HBM bandwidth (roofline ~358 GB/s/core ~ 375 us).
  - Pass drain: ScalarE activation(Copy, accum_out) evacuates each
    psum bank to SBUF and emits the Gamma partial (row-sum) in the
    same instruction, freeing the bank for the next pass at ~0.7 us
    per bank instead of a serial VectorE drain of PSUM.  VectorE then
    does a fused multiply+reduce (tensor_tensor_reduce) against the
    packed Y^T [16, N] tile from the SBUF copy, overlapped with the
    next pass's matmuls/DMAs.
  - Host sums the tiny per-core partials and adds the Y-only balance
    term.
"""

import sys

if "/opt/trn_rl_repo" not in sys.path:
    sys.path.insert(0, "/opt/trn_rl_repo")

import numpy as np

N = 16384
G = 16
NC = 8
R = N // NC            # 2048 rows of A per core
KT = R // 128          # 16 row-blocks per core
QP = 4                 # column quarter-passes
CN = N // QP           # 4096 columns per pass
JT = CN // 512         # 8 column tiles of 512 per pass (one psum bank each)

_NC_CACHE = None
last_results = None    # BassKernelResults of the most recent run


def _build():
    import concourse.mybir as mybir
    from concourse.bacc import Bacc
    from concourse.bass import MemorySpace, ds
    from concourse.tile import TileContext

    f32 = mybir.dt.float32
    f32r = mybir.dt.float32r

    nc = Bacc(trn_type="TRN2")
    a_d = nc.declare_dram_parameter("A", [R, N], f32r, isOutput=False)
    yl_d = nc.declare_dram_parameter("Ylp", [128, KT, G], f32r, isOutput=False)
    yt_d = nc.declare_dram_parameter("YT", [G, N], f32, isOutput=False)
    out_d = nc.declare_dram_parameter("out", [G, 2 * QP * JT], f32, isOutput=True)

    with TileContext(nc) as tc:
        with (
            tc.tile_pool(name="const", bufs=1) as cpool,
            tc.tile_pool(name="abuf", bufs=6) as apool,
            tc.tile_pool(name="bcopy", bufs=2) as bpool,
            tc.tile_pool(name="scr", bufs=2) as spool,
            tc.tile_pool(name="psum", bufs=1, space=MemorySpace.PSUM) as ppool,
        ):
            yl = cpool.tile([128, KT, G], f32r)
            nc.sync.dma_start(out=yl, in_=yl_d[:])
            # Y^T packed on 16 partitions (group i on partition i), full
            # free dim -- all DVE operands share partition base 0.
            yt = cpool.tile([G, N], f32)
            nc.sync.dma_start(out=yt, in_=yt_d[:])
            gs = cpool.tile([G, 2 * QP * JT], f32)

            psums = [
                ppool.tile([128, 512], f32, name=f"psum{b}") for b in range(JT)
            ]

            for p in range(QP):
                for k in range(KT):
                    ak = apool.tile([128, CN], f32r)
                    nc.sync.dma_start(
                        out=ak, in_=a_d[ds(k * 128, 128), ds(p * CN, CN)]
                    )
                    for jj in range(JT):
                        nc.tensor.matmul(
                            psums[jj][ds(0, G), :],
                            yl[:, k, :],
                            ak[:, ds(512 * jj, 512)],
                            start=(k == 0),
                            stop=(k == KT - 1),
                        )

                # Drain: ScalarE copies each bank to SBUF and emits the
                # Gamma partial (free-dim row sum) via accum_out. This
                # frees psum bank jj for pass p+1 quickly.
                bc = bpool.tile([G, JT, 512], f32)
                for jj in range(JT):
                    nc.scalar.activation(
                        out=bc[:, jj, :],
                        in_=psums[jj][ds(0, G), :],
                        func=mybir.ActivationFunctionType.Copy,
                        accum_out=gs[:, ds(p * JT + jj, 1)],
                    )
                # Background: fused (B * Y^T) multiply + row-sum from the
                # SBUF copy (overlaps the next pass).
                # (tensor_tensor_reduce faults at runtime on HW, even from
                # SBUF; scalar_tensor_tensor's accum_out path works.)
                for jj in range(JT):
                    scratch = spool.tile([G, 512], f32)
                    nc.vector.scalar_tensor_tensor(
                        out=scratch,
                        in0=bc[:, jj, :],
                        scalar=1.0,
                        in1=yt[:, ds((p * JT + jj) * 512, 512)],
                        op0=mybir.AluOpType.mult,
                        op1=mybir.AluOpType.mult,
                        accum_out=gs[:, ds(QP * JT + p * JT + jj, 1)],
                    )

            nc.sync.dma_start(out=out_d[:], in_=gs)

    nc.finalize()  # Bacc: runs wait-splitting (generate_event_semaphores) + reg alloc
    return nc


def _get_nc():
    global _NC_CACHE
    if _NC_CACHE is None:
        _NC_CACHE = _build()
    return _NC_CACHE


def _pack_inputs(Y, A):
    """Host-side packed layouts so the device does zero reshuffling."""
    # Ylp[c][p, k, i] = Y[c*R + k*128 + p, i]  (matmul lhsT per row-block)
    ylp = Y.reshape(NC, KT, 128, G).transpose(0, 2, 1, 3).copy()
    ytp = np.ascontiguousarray(Y.T)  # [G, N]
    in_maps = [
        {"A": np.ascontiguousarray(A[c * R : (c + 1) * R]), "Ylp": ylp[c], "YT": ytp}
        for c in range(NC)
    ]
    return in_maps


def kernel(Y, A, _trace=False, _trace_kwargs=None):
    global last_results
    Y = np.asarray(Y, dtype=np.float32)
    A = np.asarray(A, dtype=np.float32)
    assert Y.shape == (N, G) and A.shape == (N, N)

    from concourse.bass_utils import run_bass_kernel_spmd

    in_maps = _pack_inputs(Y, A)
    res = run_bass_kernel_spmd(
        _get_nc(),
        in_maps,
        core_ids=list(range(NC)),
        trace=_trace,
        **(_trace_kwargs or {}),
    )
    last_results = res

    g_total = np.zeros(G, dtype=np.float64)
    s_total = np.zeros(G, dtype=np.float64)
    for c in range(NC):
        o = np.asarray(res.results[c]["out"], dtype=np.float64)  # [G, 2*QP*JT]
        g_total += o[:, : QP * JT].sum(axis=1)
        s_total += o[:, QP * JT :].sum(axis=1)

    gamma = g_total
    numer = gamma - s_total
    cut = float(np.sum(numer / gamma))
    col = Y.sum(axis=0, dtype=np.float64)
    balance = float(np.sum((col - N / G) ** 2))
    return np.float32(cut + balance)
